# revision 38
# baseline (speedup 1.0000x reference)
"""Trainium2 Bass kernel for CustomizablePatchDominantGradientOrientation.

Pipeline per patch (32x32, fp32):
  sobel (replicate pad, [1,2,1]x[-1,0,1] separable; /8 dropped - the final
  angle is invariant to a global scale on (gx, gy, mag));
  mag = sqrt(gx^2+gy^2) via a fused square-sum custom op + ACT Sqrt;
  o = 36-bin-unit orientation in [0, 36]: ACT Arctan on gy/gx, then a fused
  scale+quadrant-select custom op (o = (18/pi)*atan + {0|18|36});
  36-bin soft (tent) histogram via RAMP DECOMPOSITION: the tent mass
  T_k = sum_pix m*relu(1-|o-k|) is a second difference of 37 ramp sums
  (down ramps h_j = sum m*relu(j-o), j=1..18; up ramps
  g_j = sum m*relu(o-j), j=17..35; wrap bin T_0 = h_1 + g_35).  Each ramp
  is 4 DVE ALU stages (sub, relu, mult, scan-add), so a HAND-BUILT custom
  DVE op (raw uop table, not the Spec DSL) computes TWO ramps per pass:
  accumulator A at stage 3 streamed out through a delay-lane with a
  stride-0-collapsed output (last element's write survives = final sum),
  accumulator B at stage 7 via the standard accumulator-readback path
  (accum_enabled + out_a flop -> accum_out[P,1]).  19 passes/tile replace
  the 36 tent-scan passes of the one-bin-per-pass formulation (measured
  identical per-element rate, 1 elem/cycle fp32).
  circular smoothing, argmax, parabolic refinement -> angle.

Engine placement (all measured): SOBEL ON THE TENSOR ENGINE - each tile
is PE-transposed (identity matmul) chunk-by-chunk into [px, b] layout,
then gx/gy come from banded Kronecker-block matmuls (5 distinct [128,128]
fp32 blocks per gradient, shipped as constants; fp32 matmuls run as HI/LO
instruction pairs; K<128 partition-sliced and bf16-moving variants were
both tried and are SLOWER/rejected).  ACT does the PSUM->SBUF copies
(+1e-18 bias fold), sqrt, arctan, and tail copies.  Everything else on
the Vector engine (DVE): the histogram ramp passes, gx^2+gy^2, the
reciprocal (gy/gx via reciprocal_approx_fast + multiply - a fused
single-op Halley variant was tried and reverted: its 2e-4 error flips ~11
argmax near-ties vs 3e-6 for the 2-op chain), and the quadrant op.
GPSIMD shares SBUF ports with DVE on TRN2 and inflates DVE op durations -
unused.  DVE is ~100% busy; PE ~90%; ACT ~25%.

Data parallel: B=32768 patches sharded over 8 NeuronCores (4096 each);
per core 32 tiles of [128 patches x 1024 pixels], grouped 4 tiles per
ACT-table-set phase group; the histogram of group g is emitted after the
phases of group g+1, and the tail runs in two halves (first half overlaps
the later groups' histogram passes, with an early half-output DMA).
"""

import math

import numpy as np

NBINS = 36
PI = math.pi
PATCH = 32
HW = PATCH * PATCH
P = 128          # partitions (patches per tile)
N_CORES = 8
GROUP = 4        # tiles per ACT-phase group
NRAMP_OPS = 19   # ceil(38/2): ramp slots [h_0..h_18, g_17..g_35]

_BUILD_CACHE = {}
_OPS_REGISTERED = {}


# --------------------------------------------------------------------------
# hand-built dual-ramp custom DVE op (2 ramp sums per 1-elem/cycle pass)
# --------------------------------------------------------------------------
def _build_ramp2_uops(kindA, kindB):
    from concourse.dve_uop import (
        AluInp, AluOp, DelayInp, InpSel, OutPath, OutSel, Trigger, UopConfig,
    )

    # lanes: 0=Src0(o), 1=C0(jA), 2=C1(jB), 3=ZERO, 4=Src1(m), 5=accA capture
    L_O, L_CA, L_CB, L_Z, L_M, L_ACC = 0, 1, 2, 3, 4, 5
    D = AluInp.PREV_DELAY_0

    def mk():
        u = UopConfig()
        u.enable_input(InpSel.SRC_0, L_O + 1)
        u.enable_input(InpSel.CONST_0, L_CA + 1)
        u.enable_input(InpSel.CONST_1, L_CB + 1)
        u.enable_input(InpSel.ZERO, L_Z + 1)
        u.enable_input(InpSel.SRC_1, L_M + 1)
        for st in range(8):
            dp = u.datapath_config[st]
            dp.pass_through_delay(L_O, L_CA, L_CB, L_Z, L_M)
            if st >= 4:
                dp.pass_through_delay(L_ACC)
        dps = u.datapath_config
        if kindA == "down":
            dps[0].enable_alu(AluOp.SUBTRACT, AluInp(D + L_CA), AluInp(D + L_O))
        else:
            dps[0].enable_alu(AluOp.SUBTRACT, AluInp(D + L_O), AluInp(D + L_CA))
        dps[1].enable_alu(AluOp.MAX, AluInp.PREV_ALU_OUT, AluInp(D + L_Z))
        dps[2].enable_alu(AluOp.MULTIPLY, AluInp.PREV_ALU_OUT, AluInp(D + L_M))
        dps[3].enable_alu(AluOp.ADD, AluInp.CURR_ALU_OUT, AluInp.PREV_ALU_OUT)
        if kindB == "down":
            dps[4].enable_alu(AluOp.SUBTRACT, AluInp(D + L_CB), AluInp(D + L_O))
        else:
            dps[4].enable_alu(AluOp.SUBTRACT, AluInp(D + L_O), AluInp(D + L_CB))
        dps[4].enable_delay_from_src(DelayInp.PREV_ALU_OUT, L_ACC)
        dps[5].enable_alu(AluOp.MAX, AluInp.PREV_ALU_OUT, AluInp(D + L_Z))
        dps[6].enable_alu(AluOp.MULTIPLY, AluInp.PREV_ALU_OUT, AluInp(D + L_M))
        dps[7].enable_alu(AluOp.ADD, AluInp.CURR_ALU_OUT, AluInp.PREV_ALU_OUT)
        u.require_inp0 = 1
        u.require_inp1 = 1
        u.accum_enabled = 1
        dps[7].alu_out_a_enable = 1
        return u

    seed = mk()
    seed.require_inp0 = 0
    seed.require_inp1 = 0
    seed.datapath_config[3].enable_alu(
        AluOp.BYPASS, AluInp(D + L_Z), AluInp(D + L_Z))
    seed.datapath_config[7].enable_alu(
        AluOp.BYPASS, AluInp(D + L_Z), AluInp(D + L_Z))
    seed.datapath_config[7].alu_out_a_enable = 1
    seed.repeat_count = 1
    seed.trigger = (Trigger.COUNT, Trigger.NONE, Trigger.NONE)
    seed.next_uop = (1, 0, 0)

    steady = mk()
    steady.trigger = (Trigger.SRC_TENSOR_DONE, Trigger.NONE, Trigger.NONE)
    steady.next_uop = (0, 0, 0)
    steady.enable_output(OutSel.DELAY_5, OutPath.WR0_LO)
    # keep control_slow identical across the chain (required by HW)
    seed.out[OutPath.WR0_LO] = OutSel.DELAY_5
    return [seed, steady]


def _ramp_np(o, m, j, kind):
    d = (j - o) if kind == "down" else (o - j)
    return (np.maximum(d, 0.0) * m).sum(axis=-1, keepdims=True)


def _register_custom_ops():
    """Register fused DSL ops + the hand-built dual-ramp ops at runtime."""
    if _OPS_REGISTERED:
        return _OPS_REGISTERED

    from operator import add as _opadd

    import concourse.dve_ops as dve_ops
    from concourse.dve_ops import DveOp
    from concourse.dve_spec import (
        Spec, Src0, Src1, C0, C1, C2, Zero, relu, select, scan, sq,
        AluOp, lower, _has_src1,
    )
    from concourse.dve_uop import DveOpSpec

    def _reg_dsl(name, spec):
        if name in dve_ops._SUB_OPCODE_FOR_NAME:
            for op in dve_ops.OPS:
                if op.name == name:
                    return op
        row = dve_ops._CUSTOM_DVE_ROW_BASE + len(dve_ops.OPS)
        assert row < 0x20, "custom-DVE row budget exhausted"
        dve_ops._SUB_OPCODE_FOR_NAME[name] = row
        shas = {}
        for ver in ("v3", "v4"):
            s = DveOpSpec(name=name, opcode=row, uops=lower(spec, ver=ver),
                          rd1_en=_has_src1(spec))
            shas[ver] = s.sha(ver)
        op = DveOp(name, spec, subdim=False, uops_sha=shas)
        dve_ops.OPS.append(op)
        dve_ops.CUSTOM_DVE_SPECS[name] = spec
        return op

    def _reg_hand(name, uops, fake_spec):
        if name in dve_ops._SUB_OPCODE_FOR_NAME:
            for op in dve_ops.OPS:
                if op.name == name:
                    return op
        row = dve_ops._CUSTOM_DVE_ROW_BASE + len(dve_ops.OPS)
        assert row < 0x20, "custom-DVE row budget exhausted"
        spec_obj = DveOpSpec(name=name, opcode=row, uops=uops, rd1_en=True)
        spec_obj.validate("v3")
        shas = {ver: spec_obj.sha(ver) for ver in ("v3", "v4")}
        dve_ops._SUB_OPCODE_FOR_NAME[name] = row
        op = DveOp(name, fake_spec, subdim=False, uops_sha=shas)
        dve_ops.OPS.append(op)
        dve_ops.CUSTOM_DVE_SPECS[name] = fake_spec
        dve_ops._COMPILE_CACHE[(name, "v3")] = spec_obj
        dve_ops._COMPILE_CACHE[(name, "v4")] = spec_obj
        return op

    # --- DSL ops ---
    def _gsq_ref(in0, in1, s0, s1, imm2):
        return (in0 * in0 + in1 * in1).astype(np.float32)

    _OPS_REGISTERED["gsq"] = _reg_dsl(
        "GRAD_SQSUM_ANT",
        Spec(body=sq(Src0) + sq(Src1), reference=_gsq_ref))

    # o = c*au + (gxs<0 ? (au>=0 ? 0 : 36) : 18)   [in0=au, in1=gxs,
    #  s0=c=18/pi, s1=18, imm2=36]; o in [0,36], bin-unit orientation.
    def _quadsc_ref(in0, in1, s0, s1, imm2):
        adj = np.where(in1 < 0, np.where(in0 >= 0, 0.0, imm2), s1)
        return (in0 * s0 + adj).astype(np.float32)

    _OPS_REGISTERED["quadsc"] = _reg_dsl(
        "QUAD_SCALE_ANT",
        Spec(body=Src0 * C0 + select(Src1 < Zero,
                                     select(Src0 >= Zero, Zero, C2), C1),
             reference=_quadsc_ref))

    # u = in1/in0 in ONE 8-stage op: bit-flip seed (x*bitcast(~x) lands in
    # [-4.5,-4]), Chebyshev scale c0, one Halley step (cubic: 6% -> 2e-4),
    # times in1.  [s0=-0.23549792, s1=3.0]
    def _rcpmul_ref(in0, in1, s0, s1, imm2):
        nx = (~in0.view(np.int32)).view(np.float32)
        y0 = nx * np.float32(s0 if not isinstance(s0, np.ndarray) else s0[0])
        t = in0 * y0
        return (((t - 3.0) * t + 3.0) * y0 * in1).astype(np.float32)

    from concourse.dve_spec import Bin as _Bin
    _ny0 = _Bin(AluOp.BITWISE_NOT, Src0, Src0) * C0
    _t = Src0 * _ny0
    _OPS_REGISTERED["rcpmul"] = _reg_dsl(
        "RECIP_MUL_HALLEY_ANT",
        Spec(body=((_t - C1) * _t + C1) * _ny0 * Src1,
             reference=_rcpmul_ref))

    # --- hand-built dual-ramp ops ---
    def _fake_ramp2(kindA, kindB):
        def rampexpr(c, kind):
            return (relu(c - Src0) if kind == "down" else relu(Src0 - c)) * Src1

        def ref(in0, in1, s0, s1, imm2, kA=kindA, kB=kindB):
            p = in0.shape[0]
            o = in0.reshape(p, -1).astype(np.float64)
            m = in1.reshape(p, -1).astype(np.float64)
            a = _ramp_np(o, m, s0, kA)
            b = _ramp_np(o, m, s1, kB)
            return (np.broadcast_to(a, o.shape).astype(np.float32),
                    b.astype(np.float32))

        return Spec(body=scan(AluOp.ADD, rampexpr(C0, kindA)), accum=_opadd,
                    reference=ref)

    for key, kA, kB in (("ramp_dd", "down", "down"),
                        ("ramp_du", "down", "up"),
                        ("ramp_uu", "up", "up")):
        _OPS_REGISTERED[key] = _reg_hand(
            f"RAMP2_{kA[0].upper()}{kB[0].upper()}_ANT",
            _build_ramp2_uops(kA, kB), _fake_ramp2(kA, kB))
    return _OPS_REGISTERED


def _make_kmat():
    """11 [128,128] blocks: sobel Kronecker factors for the PE path.
    K[px_in, px_out] = Av[r,r']*Dh[c,c'] (gx) / Dv[r,r']*Ah[c,c'] (gy) with
    replicate padding; coupling spans one 128-px chunk, with only 5 distinct
    blocks per gradient (diag d=0 / middle / d=7, super-, sub-diagonal)."""
    def band(w):
        A = np.zeros((32, 32), dtype=np.float64)
        for cp in range(32):
            for dc, wt in zip((-1, 0, 1), w):
                A[min(max(cp + dc, 0), 31), cp] += wt
        return A

    sm, df = band((1.0, 2.0, 1.0)), band((-1.0, 0.0, 1.0))
    blocks = []
    for K in (np.kron(sm, df), np.kron(df, sm)):
        b = K.reshape(8, 128, 8, 128)
        blocks += [b[0, :, 0, :], b[1, :, 1, :], b[7, :, 7, :],
                   b[0, :, 1, :], b[1, :, 0, :]]
    blocks.append(np.eye(128))
    return np.ascontiguousarray(
        np.concatenate(blocks, axis=1).astype(np.float32))


# ramp slot r (0..37) -> (kind, threshold j): slots 0..18 = h_0..h_18 (down),
# slots 19..37 = g_17..g_35 (up).  Pair op i covers slots (2i, 2i+1).
def _ramp_slot(r):
    return ("down", float(r)) if r <= 18 else ("up", float(r - 2))


def _pair_spec(i):
    kA, jA = _ramp_slot(2 * i)
    kB, jB = _ramp_slot(2 * i + 1)
    return kA, jA, kB, jB


# --------------------------------------------------------------------------
# kernel build
# --------------------------------------------------------------------------
def _build(b_core, smooth_w, wk_is_ones):
    import concourse.bacc as bacc
    import concourse.mybir as mybir
    from concourse.tile import TileContext
    from concourse.bass import broadcast_tensor_aps

    ops = _register_custom_ops()
    QUADSC, GSQ, RCPMUL = ops["quadsc"], ops["gsq"], ops["rcpmul"]
    RAMP_OPS = {("down", "down"): ops["ramp_dd"],
                ("down", "up"): ops["ramp_du"],
                ("up", "up"): ops["ramp_uu"]}

    f32 = mybir.dt.float32
    Alu = mybir.AluOpType
    Act = mybir.ActivationFunctionType

    n_tiles = b_core // P
    assert b_core % P == 0 and n_tiles % GROUP == 0
    w0, w1, w2 = (float(x) for x in smooth_w)

    nc = bacc.Bacc(None, target_bir_lowering=False, debug=False)
    patch_in = nc.dram_tensor("patch", [b_core, HW], f32, kind="ExternalInput")
    # consts: iota36 repeated n_tiles times, then (iota36 - 64) repeated
    consts_in = nc.dram_tensor("consts", [P, 2 * n_tiles * NBINS], f32,
                               kind="ExternalInput")
    # sobel Kronecker blocks for the PE path: [gx d0,dM,d7,sup,sub;
    # gy d0,dM,d7,sup,sub; identity] as 11 [128,128] blocks
    kmat_in = nc.dram_tensor("kmat", [P, 11 * 128], f32, kind="ExternalInput")
    wk_in = None
    if not wk_is_ones:
        wk_in = nc.dram_tensor("wk", [P, HW], f32, kind="ExternalInput")
    out_t = nc.dram_tensor("angle", [b_core], f32, kind="ExternalOutput")

    with TileContext(nc) as tc:
        with tc.tile_pool(name="pool", bufs=2) as pool, \
             tc.tile_pool(name="psumq", bufs=1, space="PSUM") as pq, \
             tc.tile_pool(name="persist", bufs=1) as pp:
            IOTA = pp.tile([P, 1, NBINS], f32)
            IOTA64 = pp.tile([P, 1, NBINS], f32)
            KM = pp.tile([P, 11 * 128], f32)
            nc.gpsimd.dma_start(KM[:], kmat_in[:])
            ID128 = KM[:, 10 * 128:11 * 128]
            WK = None
            if wk_in is not None:
                WK = pp.tile([P, HW], f32)
                nc.sync.dma_start(WK[:], wk_in[:])

            # 39 ramp slots per tile: [h_0..h_18, g_17..g_35, zero-pad]
            RAMP = pp.tile([P, n_tiles, 39], f32)
            ANG = pp.tile([P, n_tiles], f32)

            n_groups = n_tiles // GROUP

            def emit_hist(hg, hAW, hMW):
                for s in range(GROUP):
                    t = hg * GROUP + s
                    if t == 0:
                        continue
                    o_ap = hAW[:, s, :]
                    m_ap = hMW[:, s, :]
                    for i in range(NRAMP_OPS):
                        kA, jA, kB, jB = _pair_spec(i)
                        nc.vector._custom_dve(
                            RAMP_OPS[(kA, kB)],
                            out=RAMP[:, t, 2 * i:2 * i + 1]
                                .broadcast_to([P, HW]),
                            in0=o_ap, in1=m_ap, s0=jA, s1=jB,
                            accum_out=RAMP[:, t, 2 * i + 1:2 * i + 2])

            # ---- tail: ramps -> tents -> smoothing, argmax, refinement ----
            # T_k = 2nd difference of ramp sums; wrap bin T_0 = h_1 + g_35.
            # HEXT holds [T_35, T_0..T_35, T_0] (circular padding built in).
            HEXT = pp.tile([P, n_tiles, NBINS + 2], f32)
            SM = pp.tile([P, n_tiles, NBINS], f32)
            HS = pp.tile([P, n_tiles, NBINS], f32)
            VMAX = pp.tile([P, n_tiles, 1], f32)
            EQ = pp.tile([P, n_tiles, NBINS], f32)
            IDX = pp.tile([P, n_tiles, 1], f32)
            NV = {nm: [pp.tile([P, n_tiles, 1], f32, name=f"idxn_{nm}"),
                       pp.tile([P, n_tiles, 1], f32, name=f"wadj_{nm}"),
                       pp.tile([P, n_tiles, NBINS], f32, name=f"dif_{nm}"),
                       pp.tile([P, n_tiles, 1], f32, name=f"v_{nm}")]
                  for nm in ("p", "m")}
            NUM = pp.tile([P, n_tiles, 1], f32)
            SUMN = pp.tile([P, n_tiles, 1], f32)
            DEN = pp.tile([P, n_tiles, 1], f32)
            RECD = pp.tile([P, n_tiles, 1], f32)
            SCD = pp.tile([P, n_tiles, 1], f32)
            REF = pp.tile([P, n_tiles, 1], f32)
            out_view = out_t[:].rearrange("(t p) -> p t", p=P)

            def emit_tail(lo, hi):
                nt = hi - lo
                R = RAMP[:, lo:hi, :]
                HX = HEXT[:, lo:hi, :]
                # k=1..17: T_k = R[k-1] - 2R[k] + R[k+1]   -> HEXT[, k+1]
                nc.vector.tensor_tensor(HX[:, :, 2:19], R[:, :, 0:17],
                                        R[:, :, 2:19], Alu.add)
                nc.vector.scalar_tensor_tensor(
                    out=HX[:, :, 2:19], in0=R[:, :, 1:18], scalar=-2.0,
                    in1=HX[:, :, 2:19], op0=Alu.mult, op1=Alu.add)
                # k=18..35: T_k = R[k+1] - 2R[k+2] + R[k+3] -> HEXT[, k+1]
                nc.vector.tensor_tensor(HX[:, :, 19:37], R[:, :, 19:37],
                                        R[:, :, 21:39], Alu.add)
                nc.vector.scalar_tensor_tensor(
                    out=HX[:, :, 19:37], in0=R[:, :, 20:38], scalar=-2.0,
                    in1=HX[:, :, 19:37], op0=Alu.mult, op1=Alu.add)
                # T_0 = h_1 + g_35 = RAMP[1] + RAMP[37]
                nc.vector.tensor_tensor(HX[:, :, 1:2], R[:, :, 1:2],
                                        R[:, :, 37:38], Alu.add)
                # circular pads (single-source copies on ACT)
                nc.scalar.activation(HX[:, :, 0:1], HX[:, :, 36:37], Act.Copy)
                nc.scalar.activation(HX[:, :, 37:38], HX[:, :, 1:2], Act.Copy)

                sm = SM[:, lo:hi, :]
                nc.scalar.activation(sm, HX[:, :, 2:38], Act.Copy, scale=w2)
                nc.vector.scalar_tensor_tensor(
                    out=sm, in0=HX[:, :, 0:36], scalar=w0, in1=sm,
                    op0=Alu.mult, op1=Alu.add)
                hs = HS[:, lo:hi, :]
                nc.vector.scalar_tensor_tensor(
                    out=hs, in0=HX[:, :, 1:37], scalar=w1, in1=sm,
                    op0=Alu.mult, op1=Alu.add)

                vmax = VMAX[:, lo:hi, :]
                nc.vector.tensor_reduce(vmax, hs, mybir.AxisListType.X,
                                        Alu.max)
                eq = EQ[:, lo:hi, :]
                hs_b, vmax_b = broadcast_tensor_aps(hs, vmax)
                nc.vector.tensor_tensor(eq, hs_b, vmax_b, Alu.is_equal)
                nc.vector.tensor_tensor(
                    eq, eq, IOTA64[:].broadcast_to([P, nt, NBINS]), Alu.mult)
                idx = IDX[:, lo:hi, :]
                nc.vector.tensor_reduce(idx, eq, mybir.AxisListType.X,
                                        Alu.min)
                nc.scalar.activation(idx, idx, Act.Copy, bias=64.0)

                def neighbor_value(shift, wrap_thr, wrap_add, nm):
                    IDXN, WADJ, DIF, V = (x[:, lo:hi, :] for x in NV[nm])
                    nc.scalar.activation(IDXN, idx, Act.Copy,
                                         bias=float(shift))
                    if wrap_add < 0:
                        nc.vector.tensor_scalar(WADJ, IDXN, wrap_thr,
                                                float(wrap_add), Alu.is_gt,
                                                Alu.mult)
                    else:
                        nc.vector.tensor_scalar(WADJ, IDXN, wrap_thr,
                                                float(wrap_add), Alu.is_lt,
                                                Alu.mult)
                    nc.vector.tensor_tensor(IDXN, IDXN, WADJ, Alu.add)
                    iota_b, idxn_b = broadcast_tensor_aps(
                        IOTA[:].broadcast_to([P, nt, NBINS]), IDXN)
                    nc.vector.tensor_tensor(DIF, iota_b, idxn_b,
                                            Alu.subtract)
                    nc.vector.tensor_scalar(DIF, DIF, 0.0, None, Alu.is_equal)
                    nc.vector.tensor_tensor(DIF, DIF, hs, Alu.mult)
                    nc.vector.tensor_reduce(V, DIF, mybir.AxisListType.X,
                                            Alu.add)
                    return V

                VP = neighbor_value(+1, 35.5, -36.0, "p")
                VM = neighbor_value(-1, -0.5, +36.0, "m")

                num = NUM[:, lo:hi, :]
                nc.vector.tensor_tensor(num, VP, VM, Alu.subtract)
                sumn = SUMN[:, lo:hi, :]
                nc.vector.tensor_tensor(sumn, VP, VM, Alu.add)
                den = DEN[:, lo:hi, :]
                nc.scalar.activation(den, vmax, Act.Copy, scale=2.0)
                nc.vector.tensor_tensor(den, den, sumn, Alu.subtract)
                recd = RECD[:, lo:hi, :]
                nc.vector.reciprocal_approx_accurate(recd, den,
                                                     SCD[:, lo:hi, :])
                ref = REF[:, lo:hi, :]
                nc.vector.scalar_tensor_tensor(
                    out=ref, in0=num, scalar=0.5, in1=recd,
                    op0=Alu.mult, op1=Alu.mult)
                nc.vector.tensor_tensor(ref, idx, ref, Alu.add)
                nc.vector.tensor_scalar(ANG[:, lo:hi], ref[:, :, 0],
                                        -2.0 * PI / NBINS, PI,
                                        Alu.mult, Alu.add)
                nc.sync.dma_start(out_view[:, lo:hi], ANG[:, lo:hi])


            pending = None  # (g, AW, MW) - hist deferred one group so the
            # next group's phases are already queued when DVE grinds the
            # ramp ops (keeps every engine fed across group boundaries)
            for g in range(n_groups):
                tiles = range(g * GROUP, (g + 1) * GROUP)
                AW = pool.tile([P, GROUP, HW], f32, tag="aw", name=f"aw{g}")
                MW = pool.tile([P, GROUP, HW], f32, tag="mw", name=f"mw{g}")
                slot = {}
                # ---- phase A: sobel on PE (transpose + banded Kronecker
                # matmuls), mag (ACT sqrt table) ----
                for t in tiles:
                    s = t % GROUP
                    X = pool.tile([P, HW], f32, tag="x", bufs=3, name=f"x{t}")
                    nc.sync.dma_start(X[:], patch_in[t * P:(t + 1) * P, :])

                    if t == 0:
                        # tile 0 on the otherwise-idle DVE so its pipeline
                        # starts ~15us before the PE chain warms up
                        X3 = X.rearrange("p (r c) -> p r c", c=PATCH)
                        SV = pool.tile([P, HW], f32, tag="sv", name=f"sv{t}")
                        T1 = pool.tile([P, HW], f32, tag="t1", bufs=1,
                                       name=f"t1{t}")
                        nc.vector.tensor_tensor(
                            T1[:, 0:992], X[:, 0:992], X[:, 32:1024], Alu.add)
                        nc.vector.tensor_tensor(
                            SV[:, 32:992], T1[:, 0:960], T1[:, 32:992],
                            Alu.add)
                        SVE = SV.rearrange("p (r c) -> p r c", c=PATCH)[:, 0:32:31, :]
                        T1E = T1.rearrange("p (r c) -> p r c", c=PATCH)[:, 0:31:30, :]
                        nc.vector.scalar_tensor_tensor(
                            out=SVE, in0=X3[:, 0:32:31, :], scalar=2.0,
                            in1=T1E, op0=Alu.mult, op1=Alu.add)
                        SV3 = SV.rearrange("p (r c) -> p r c", c=PATCH)
                        GX = pool.tile([P, HW], f32, tag="gx", bufs=4,
                                       name=f"gx{t}")
                        GX3 = GX.rearrange("p (r c) -> p r c", c=PATCH)
                        nc.vector.tensor_tensor(
                            GX[:, 1:1023], SV[:, 2:1024], SV[:, 0:1022],
                            Alu.subtract)
                        nc.vector.tensor_tensor(
                            GX3[:, :, 0:32:31], SV3[:, :, 1:32:30],
                            SV3[:, :, 0:31:30], Alu.subtract)
                        SH = pool.tile([P, HW], f32, tag="sh", bufs=4,
                                       name=f"sh{t}")
                        SH3 = SH.rearrange("p (r c) -> p r c", c=PATCH)
                        nc.vector.scalar_tensor_tensor(
                            out=SH[:, 1:1023], in0=X[:, 1:1023], scalar=2.0,
                            in1=X[:, 0:1022], op0=Alu.mult, op1=Alu.add)
                        nc.vector.tensor_tensor(
                            SH[:, 1:1023], SH[:, 1:1023], X[:, 2:1024],
                            Alu.add)
                        nc.vector.scalar_tensor_tensor(
                            out=SH3[:, :, 0:32:31], in0=X3[:, :, 0:32:31],
                            scalar=3.0, in1=X3[:, :, 1:31:29], op0=Alu.mult,
                            op1=Alu.add)
                        GY = pool.tile([P, HW], f32, tag="gy", bufs=4,
                                       name=f"gy{t}")
                        nc.vector.tensor_tensor(
                            GY[:, 32:992], SH[:, 64:1024], SH[:, 0:960],
                            Alu.subtract)
                        GYE = GY.rearrange("p (r c) -> p r c", c=PATCH)[:, 0:32:31, :]
                        SH3e = SH.rearrange("p (r c) -> p r c", c=PATCH)
                        nc.vector.tensor_tensor(
                            GYE, SH3e[:, 1:32:30, :], SH3e[:, 0:31:30, :],
                            Alu.subtract)
                        nc.scalar.activation(GX[:], GX[:], Act.Copy,
                                             bias=1e-18)
                        if WK is not None:
                            nc.vector.tensor_tensor(GX[:], GX[:], WK[:],
                                                    Alu.mult)
                            nc.vector.tensor_tensor(GY[:], GY[:], WK[:],
                                                    Alu.mult)
                        G2 = pool.tile([P, HW], f32, tag="g2", name=f"g2{t}")
                        nc.vector._custom_dve(GSQ, out=G2[:], in0=GX[:],
                                              in1=GY[:])
                        nc.scalar.activation(MW[:, s, :], G2[:], Act.Sqrt)
                        RC = pool.tile([P, HW], f32, tag="rc", bufs=4,
                                       name=f"rc{t}")
                        nc.vector.reciprocal_approx_fast(RC[:], GX[:])
                        # run tile 0's phase B + histogram inline: ~30us of
                        # immediate DVE work covering the PE/ACT warmup
                        U0 = pool.tile([P, HW], f32, tag="sv", name="u0w")
                        nc.vector.tensor_tensor(U0[:], GY[:], RC[:], Alu.mult)
                        AU0 = pool.tile([P, HW], f32, tag="sh", bufs=4,
                                        name="au0w")
                        nc.scalar.activation(AU0[:], U0[:], Act.Arctan)
                        nc.vector._custom_dve(QUADSC, out=AW[:, s, :],
                                              in0=AU0[:], in1=GX[:],
                                              s0=18.0 / PI, s1=18.0,
                                              imm2=36.0)
                        for i in range(NRAMP_OPS):
                            kA, jA, kB, jB = _pair_spec(i)
                            nc.vector._custom_dve(
                                RAMP_OPS[(kA, kB)],
                                out=RAMP[:, t, 2 * i:2 * i + 1]
                                    .broadcast_to([P, HW]),
                                in0=AW[:, s, :], in1=MW[:, s, :],
                                s0=jA, s1=jB,
                                accum_out=RAMP[:, t, 2 * i + 1:2 * i + 2])
                        continue

                    # X^T chunks via PE transpose -> PSUM -> SBUF (ACT copy)
                    XTp = pq.tile([P, HW], f32, tag="xtp", name=f"xtp{t}")
                    for ch in range(8):
                        nc.tensor.transpose(
                            XTp[:, ch * 128:(ch + 1) * 128],
                            X[:, ch * 128:(ch + 1) * 128], ID128)
                    XT = pool.tile([P, HW], f32, tag="xt", bufs=2,
                                   name=f"xt{t}")
                    nc.scalar.activation(XT[:], XTp[:], Act.Copy)

                    # G[b, px'] = sum_px X[b,px] K[px,px']: lhsT = X^T chunk,
                    # rhs = Kronecker block; accumulate over the <=3
                    # contributing input chunks per output chunk.
                    GXp = pq.tile([P, HW], f32, tag="gxp", name=f"gxp{t}")
                    GYp = pq.tile([P, HW], f32, tag="gyp", name=f"gyp{t}")
                    for c in range(8):
                        dlo, dhi = max(0, c - 1), min(7, c + 1)
                        for d in range(dlo, dhi + 1):
                            lhsT = XT[:, d * 128:(d + 1) * 128]
                            for gbase, Gp in ((0, GXp), (5, GYp)):
                                if c == d:
                                    blk = gbase + (0 if d == 0 else
                                                   (2 if d == 7 else 1))
                                elif c == d + 1:
                                    blk = gbase + 3
                                else:
                                    blk = gbase + 4
                                nc.tensor.matmul(
                                    Gp[:, c * 128:(c + 1) * 128], lhsT,
                                    KM[:, blk * 128:(blk + 1) * 128],
                                    start=(d == dlo), stop=(d == dhi))

                    # PSUM -> SBUF on ACT (frees PSUM fast; +1e-18 folded in)
                    GX = pool.tile([P, HW], f32, tag="gx", bufs=4,
                                   name=f"gx{t}")
                    nc.scalar.activation(GX[:], GXp[:], Act.Copy, bias=1e-18)
                    GY = pool.tile([P, HW], f32, tag="gy", bufs=4,
                                   name=f"gy{t}")
                    nc.scalar.activation(GY[:], GYp[:], Act.Copy)

                    if WK is not None:
                        nc.vector.tensor_tensor(GX[:], GX[:], WK[:], Alu.mult)
                        nc.vector.tensor_tensor(GY[:], GY[:], WK[:], Alu.mult)

                    # g2 = gx^2 + gy^2 fused on DVE (exact fp32; eps dropped:
                    # sqrt(0)=0 -> zero tent weight, harmless)
                    G2 = pool.tile([P, HW], f32, tag="g2", name=f"g2{t}")
                    nc.vector._custom_dve(GSQ, out=G2[:], in0=GX[:],
                                          in1=GY[:])
                    # mag feeds only the tent weights; plain ACT sqrt is ample
                    nc.scalar.activation(MW[:, s, :], G2[:], Act.Sqrt)

                    RC = pool.tile([P, HW], f32, tag="rc", bufs=4,
                                   name=f"rc{t}")
                    nc.vector.reciprocal_approx_fast(RC[:], GX[:])
                    slot[t] = (GX, RC, GY)

                # ---- phase B: orientation (ACT arctan table) ----
                slot_b = {}
                for t in tiles:
                    if t not in slot:
                        continue
                    GXS, RC, GY = slot[t]
                    U = pool.tile([P, HW], f32, tag="sv", name=f"u{t}")
                    nc.vector.tensor_tensor(U[:], GY[:], RC[:], Alu.mult)
                    AU = pool.tile([P, HW], f32, tag="sh", bufs=4,
                                   name=f"au{t}")
                    nc.scalar.activation(AU[:], U[:], Act.Arctan)
                    slot_b[t] = (GXS, AU)
                for t in tiles:
                    if t not in slot_b:
                        continue
                    s = t % GROUP
                    GXS, AU = slot_b[t]
                    # fused scale+quadrant: o = (18/pi)*au + {0|18|36}
                    nc.vector._custom_dve(QUADSC, out=AW[:, s, :],
                                          in0=AU[:], in1=GXS[:],
                                          s0=18.0 / PI, s1=18.0, imm2=36.0)

                if g == 1:
                    # tail-only constants + pad memset (needed from the
                    # first tail segment onward)
                    nc.sync.dma_start(
                        IOTA[:, 0, :], consts_in[:, 0:NBINS])
                    nc.sync.dma_start(
                        IOTA64[:, 0, :],
                        consts_in[:, n_tiles * NBINS:n_tiles * NBINS + NBINS])
                    nc.vector.memset(RAMP[:, :, 38:39], 0.0)
                if pending is not None:
                    emit_hist(*pending)
                    if pending[0] == (n_tiles // 2) // GROUP - 1:
                        emit_tail(0, n_tiles // 2)
                    elif pending[0] == n_groups - 2:
                        emit_tail(n_tiles // 2, n_tiles - GROUP)
                if g == 0:
                    # no deferral for the first group: builds DVE backlog
                    # while the PE/ACT pipeline is still filling
                    emit_hist(g, AW, MW)
                    pending = None
                else:
                    pending = (g, AW, MW)

            emit_hist(*pending)
            emit_tail(n_tiles - GROUP, n_tiles)


    nc.compile()
    return nc


def _get_built(b_core, smooth_w, wk_is_ones):
    key = (b_core, tuple(float(x) for x in smooth_w), bool(wk_is_ones))
    if key not in _BUILD_CACHE:
        _BUILD_CACHE[key] = _build(b_core, smooth_w, wk_is_ones)
    return _BUILD_CACHE[key]


# --------------------------------------------------------------------------
# host entry point
# --------------------------------------------------------------------------
def kernel(patch, weight_kernel, smooth_w):
    from concourse import bass_utils

    patch = np.ascontiguousarray(np.asarray(patch, dtype=np.float32))
    weight_kernel = np.asarray(weight_kernel, dtype=np.float32)
    smooth_w = np.asarray(smooth_w, dtype=np.float32)

    B = patch.shape[0]
    assert B % (N_CORES * P) == 0, f"B={B} not divisible by {N_CORES * P}"
    b_core = B // N_CORES
    n_tiles = b_core // P

    wk_is_ones = bool(np.all(weight_kernel == 1.0))
    nc = _get_built(b_core, smooth_w, wk_is_ones)

    x = patch.reshape(N_CORES, b_core, HW)

    iota = np.tile(np.arange(NBINS, dtype=np.float32), n_tiles)
    consts_row = np.concatenate([iota, iota - 64.0]).astype(np.float32)
    consts = np.ascontiguousarray(
        np.broadcast_to(consts_row, (P, consts_row.size)))

    kmat = _make_kmat()

    in_maps = []
    for i in range(N_CORES):
        m = {"patch": np.ascontiguousarray(x[i]), "consts": consts,
             "kmat": kmat}
        if not wk_is_ones:
            m["wk"] = np.ascontiguousarray(
                np.broadcast_to(weight_kernel.reshape(-1), (P, HW)))
        in_maps.append(m)

    res = bass_utils.run_bass_kernel_spmd(nc, in_maps,
                                          core_ids=list(range(N_CORES)))
    out = np.concatenate([r["angle"] for r in res.results])
    return out.astype(np.float32)


# revision 39
# speedup vs baseline: 1.0029x; 1.0029x over previous
"""Trainium2 Bass kernel for CustomizablePatchDominantGradientOrientation.

Pipeline per patch (32x32, fp32):
  sobel (replicate pad, [1,2,1]x[-1,0,1] separable; /8 dropped - the final
  angle is invariant to a global scale on (gx, gy, mag));
  mag = sqrt(gx^2+gy^2) via a fused square-sum custom op + ACT Sqrt;
  o = 36-bin-unit orientation in [0, 36]: ACT Arctan on gy/gx, then a fused
  scale+quadrant-select custom op (o = (18/pi)*atan + {0|18|36});
  36-bin soft (tent) histogram via RAMP DECOMPOSITION: the tent mass
  T_k = sum_pix m*relu(1-|o-k|) is a second difference of 37 ramp sums
  (down ramps h_j = sum m*relu(j-o), j=1..18; up ramps
  g_j = sum m*relu(o-j), j=17..35; wrap bin T_0 = h_1 + g_35).  Each ramp
  is 4 DVE ALU stages (sub, relu, mult, scan-add), so a HAND-BUILT custom
  DVE op (raw uop table, not the Spec DSL) computes TWO ramps per pass:
  accumulator A at stage 3 streamed out through a delay-lane with a
  stride-0-collapsed output (last element's write survives = final sum),
  accumulator B at stage 7 via the standard accumulator-readback path
  (accum_enabled + out_a flop -> accum_out[P,1]).  19 passes/tile replace
  the 36 tent-scan passes of the one-bin-per-pass formulation (measured
  identical per-element rate, 1 elem/cycle fp32).
  circular smoothing, argmax, parabolic refinement -> angle.

Engine placement (all measured): SOBEL ON THE TENSOR ENGINE - each tile
is PE-transposed (identity matmul) chunk-by-chunk into [px, b] layout,
then gx/gy come from banded Kronecker-block matmuls (5 distinct [128,128]
fp32 blocks per gradient, shipped as constants; fp32 matmuls run as HI/LO
instruction pairs; K<128 partition-sliced and bf16-moving variants were
both tried and are SLOWER/rejected).  ACT does the PSUM->SBUF copies
(+1e-18 bias fold), sqrt, arctan, and tail copies.  Everything else on
the Vector engine (DVE): the histogram ramp passes, gx^2+gy^2, the
reciprocal (gy/gx via reciprocal_approx_fast + multiply - a fused
single-op Halley variant was tried and reverted: its 2e-4 error flips ~11
argmax near-ties vs 3e-6 for the 2-op chain), and the quadrant op.
GPSIMD shares SBUF ports with DVE on TRN2 and inflates DVE op durations -
unused.  DVE is ~100% busy; PE ~90%; ACT ~25%.

Data parallel: B=32768 patches sharded over 8 NeuronCores (4096 each);
per core 32 tiles of [128 patches x 1024 pixels], grouped 4 tiles per
ACT-table-set phase group; the histogram of group g is emitted after the
phases of group g+1, and the tail runs in two halves (first half overlaps
the later groups' histogram passes, with an early half-output DMA).
"""

import math

import numpy as np

NBINS = 36
PI = math.pi
PATCH = 32
HW = PATCH * PATCH
P = 128          # partitions (patches per tile)
N_CORES = 8
GROUP = 4        # tiles per ACT-phase group
NRAMP_OPS = 19   # ceil(38/2): ramp slots [h_0..h_18, g_17..g_35]

_BUILD_CACHE = {}
_OPS_REGISTERED = {}


# --------------------------------------------------------------------------
# hand-built dual-ramp custom DVE op (2 ramp sums per 1-elem/cycle pass)
# --------------------------------------------------------------------------
def _build_ramp2_uops(kindA, kindB):
    from concourse.dve_uop import (
        AluInp, AluOp, DelayInp, InpSel, OutPath, OutSel, Trigger, UopConfig,
    )

    # lanes: 0=Src0(o), 1=C0(jA), 2=C1(jB), 3=ZERO, 4=Src1(m), 5=accA capture
    L_O, L_CA, L_CB, L_Z, L_M, L_ACC = 0, 1, 2, 3, 4, 5
    D = AluInp.PREV_DELAY_0

    def mk():
        u = UopConfig()
        u.enable_input(InpSel.SRC_0, L_O + 1)
        u.enable_input(InpSel.CONST_0, L_CA + 1)
        u.enable_input(InpSel.CONST_1, L_CB + 1)
        u.enable_input(InpSel.ZERO, L_Z + 1)
        u.enable_input(InpSel.SRC_1, L_M + 1)
        for st in range(8):
            dp = u.datapath_config[st]
            dp.pass_through_delay(L_O, L_CA, L_CB, L_Z, L_M)
            if st >= 4:
                dp.pass_through_delay(L_ACC)
        dps = u.datapath_config
        if kindA == "down":
            dps[0].enable_alu(AluOp.SUBTRACT, AluInp(D + L_CA), AluInp(D + L_O))
        else:
            dps[0].enable_alu(AluOp.SUBTRACT, AluInp(D + L_O), AluInp(D + L_CA))
        dps[1].enable_alu(AluOp.MAX, AluInp.PREV_ALU_OUT, AluInp(D + L_Z))
        dps[2].enable_alu(AluOp.MULTIPLY, AluInp.PREV_ALU_OUT, AluInp(D + L_M))
        dps[3].enable_alu(AluOp.ADD, AluInp.CURR_ALU_OUT, AluInp.PREV_ALU_OUT)
        if kindB == "down":
            dps[4].enable_alu(AluOp.SUBTRACT, AluInp(D + L_CB), AluInp(D + L_O))
        else:
            dps[4].enable_alu(AluOp.SUBTRACT, AluInp(D + L_O), AluInp(D + L_CB))
        dps[4].enable_delay_from_src(DelayInp.PREV_ALU_OUT, L_ACC)
        dps[5].enable_alu(AluOp.MAX, AluInp.PREV_ALU_OUT, AluInp(D + L_Z))
        dps[6].enable_alu(AluOp.MULTIPLY, AluInp.PREV_ALU_OUT, AluInp(D + L_M))
        dps[7].enable_alu(AluOp.ADD, AluInp.CURR_ALU_OUT, AluInp.PREV_ALU_OUT)
        u.require_inp0 = 1
        u.require_inp1 = 1
        u.accum_enabled = 1
        dps[7].alu_out_a_enable = 1
        return u

    seed = mk()
    seed.require_inp0 = 0
    seed.require_inp1 = 0
    seed.datapath_config[3].enable_alu(
        AluOp.BYPASS, AluInp(D + L_Z), AluInp(D + L_Z))
    seed.datapath_config[7].enable_alu(
        AluOp.BYPASS, AluInp(D + L_Z), AluInp(D + L_Z))
    seed.datapath_config[7].alu_out_a_enable = 1
    seed.repeat_count = 1
    seed.trigger = (Trigger.COUNT, Trigger.NONE, Trigger.NONE)
    seed.next_uop = (1, 0, 0)

    steady = mk()
    steady.trigger = (Trigger.SRC_TENSOR_DONE, Trigger.NONE, Trigger.NONE)
    steady.next_uop = (0, 0, 0)
    steady.enable_output(OutSel.DELAY_5, OutPath.WR0_LO)
    # keep control_slow identical across the chain (required by HW)
    seed.out[OutPath.WR0_LO] = OutSel.DELAY_5
    return [seed, steady]


def _ramp_np(o, m, j, kind):
    d = (j - o) if kind == "down" else (o - j)
    return (np.maximum(d, 0.0) * m).sum(axis=-1, keepdims=True)


def _register_custom_ops():
    """Register fused DSL ops + the hand-built dual-ramp ops at runtime."""
    if _OPS_REGISTERED:
        return _OPS_REGISTERED

    from operator import add as _opadd

    import concourse.dve_ops as dve_ops
    from concourse.dve_ops import DveOp
    from concourse.dve_spec import (
        Spec, Src0, Src1, C0, C1, C2, Zero, relu, select, scan, sq,
        AluOp, lower, _has_src1,
    )
    from concourse.dve_uop import DveOpSpec

    def _reg_dsl(name, spec):
        if name in dve_ops._SUB_OPCODE_FOR_NAME:
            for op in dve_ops.OPS:
                if op.name == name:
                    return op
        row = dve_ops._CUSTOM_DVE_ROW_BASE + len(dve_ops.OPS)
        assert row < 0x20, "custom-DVE row budget exhausted"
        dve_ops._SUB_OPCODE_FOR_NAME[name] = row
        shas = {}
        for ver in ("v3", "v4"):
            s = DveOpSpec(name=name, opcode=row, uops=lower(spec, ver=ver),
                          rd1_en=_has_src1(spec))
            shas[ver] = s.sha(ver)
        op = DveOp(name, spec, subdim=False, uops_sha=shas)
        dve_ops.OPS.append(op)
        dve_ops.CUSTOM_DVE_SPECS[name] = spec
        return op

    def _reg_hand(name, uops, fake_spec):
        if name in dve_ops._SUB_OPCODE_FOR_NAME:
            for op in dve_ops.OPS:
                if op.name == name:
                    return op
        row = dve_ops._CUSTOM_DVE_ROW_BASE + len(dve_ops.OPS)
        assert row < 0x20, "custom-DVE row budget exhausted"
        spec_obj = DveOpSpec(name=name, opcode=row, uops=uops, rd1_en=True)
        spec_obj.validate("v3")
        shas = {ver: spec_obj.sha(ver) for ver in ("v3", "v4")}
        dve_ops._SUB_OPCODE_FOR_NAME[name] = row
        op = DveOp(name, fake_spec, subdim=False, uops_sha=shas)
        dve_ops.OPS.append(op)
        dve_ops.CUSTOM_DVE_SPECS[name] = fake_spec
        dve_ops._COMPILE_CACHE[(name, "v3")] = spec_obj
        dve_ops._COMPILE_CACHE[(name, "v4")] = spec_obj
        return op

    # --- DSL ops ---
    def _gsq_ref(in0, in1, s0, s1, imm2):
        return (in0 * in0 + in1 * in1).astype(np.float32)

    _OPS_REGISTERED["gsq"] = _reg_dsl(
        "GRAD_SQSUM_ANT",
        Spec(body=sq(Src0) + sq(Src1), reference=_gsq_ref))

    # o = c*au + (gxs<0 ? (au>=0 ? 0 : 36) : 18)   [in0=au, in1=gxs,
    #  s0=c=18/pi, s1=18, imm2=36]; o in [0,36], bin-unit orientation.
    def _quadsc_ref(in0, in1, s0, s1, imm2):
        adj = np.where(in1 < 0, np.where(in0 >= 0, 0.0, imm2), s1)
        return (in0 * s0 + adj).astype(np.float32)

    _OPS_REGISTERED["quadsc"] = _reg_dsl(
        "QUAD_SCALE_ANT",
        Spec(body=Src0 * C0 + select(Src1 < Zero,
                                     select(Src0 >= Zero, Zero, C2), C1),
             reference=_quadsc_ref))

    # u = in1/in0 in ONE 8-stage op: bit-flip seed (x*bitcast(~x) lands in
    # [-4.5,-4]), Chebyshev scale c0, one Halley step (cubic: 6% -> 2e-4),
    # times in1.  [s0=-0.23549792, s1=3.0]
    def _rcpmul_ref(in0, in1, s0, s1, imm2):
        nx = (~in0.view(np.int32)).view(np.float32)
        y0 = nx * np.float32(s0 if not isinstance(s0, np.ndarray) else s0[0])
        t = in0 * y0
        return (((t - 3.0) * t + 3.0) * y0 * in1).astype(np.float32)

    from concourse.dve_spec import Bin as _Bin
    _ny0 = _Bin(AluOp.BITWISE_NOT, Src0, Src0) * C0
    _t = Src0 * _ny0
    _OPS_REGISTERED["rcpmul"] = _reg_dsl(
        "RECIP_MUL_HALLEY_ANT",
        Spec(body=((_t - C1) * _t + C1) * _ny0 * Src1,
             reference=_rcpmul_ref))

    # --- hand-built dual-ramp ops ---
    def _fake_ramp2(kindA, kindB):
        def rampexpr(c, kind):
            return (relu(c - Src0) if kind == "down" else relu(Src0 - c)) * Src1

        def ref(in0, in1, s0, s1, imm2, kA=kindA, kB=kindB):
            p = in0.shape[0]
            o = in0.reshape(p, -1).astype(np.float64)
            m = in1.reshape(p, -1).astype(np.float64)
            a = _ramp_np(o, m, s0, kA)
            b = _ramp_np(o, m, s1, kB)
            return (np.broadcast_to(a, o.shape).astype(np.float32),
                    b.astype(np.float32))

        return Spec(body=scan(AluOp.ADD, rampexpr(C0, kindA)), accum=_opadd,
                    reference=ref)

    for key, kA, kB in (("ramp_dd", "down", "down"),
                        ("ramp_du", "down", "up"),
                        ("ramp_uu", "up", "up")):
        _OPS_REGISTERED[key] = _reg_hand(
            f"RAMP2_{kA[0].upper()}{kB[0].upper()}_ANT",
            _build_ramp2_uops(kA, kB), _fake_ramp2(kA, kB))
    return _OPS_REGISTERED


def _make_kmat():
    """11 [128,128] blocks: sobel Kronecker factors for the PE path.
    K[px_in, px_out] = Av[r,r']*Dh[c,c'] (gx) / Dv[r,r']*Ah[c,c'] (gy) with
    replicate padding; coupling spans one 128-px chunk, with only 5 distinct
    blocks per gradient (diag d=0 / middle / d=7, super-, sub-diagonal)."""
    def band(w):
        A = np.zeros((32, 32), dtype=np.float64)
        for cp in range(32):
            for dc, wt in zip((-1, 0, 1), w):
                A[min(max(cp + dc, 0), 31), cp] += wt
        return A

    sm, df = band((1.0, 2.0, 1.0)), band((-1.0, 0.0, 1.0))
    blocks = []
    for K in (np.kron(sm, df), np.kron(df, sm)):
        b = K.reshape(8, 128, 8, 128)
        blocks += [b[0, :, 0, :], b[1, :, 1, :], b[7, :, 7, :],
                   b[0, :, 1, :], b[1, :, 0, :]]
    blocks.append(np.eye(128))
    return np.ascontiguousarray(
        np.concatenate(blocks, axis=1).astype(np.float32))


# ramp slot r (0..37) -> (kind, threshold j): slots 0..18 = h_0..h_18 (down),
# slots 19..37 = g_17..g_35 (up).  Pair op i covers slots (2i, 2i+1).
def _ramp_slot(r):
    return ("down", float(r)) if r <= 18 else ("up", float(r - 2))


def _pair_spec(i):
    kA, jA = _ramp_slot(2 * i)
    kB, jB = _ramp_slot(2 * i + 1)
    return kA, jA, kB, jB


# --------------------------------------------------------------------------
# kernel build
# --------------------------------------------------------------------------
def _build(b_core, smooth_w, wk_is_ones):
    import concourse.bacc as bacc
    import concourse.mybir as mybir
    from concourse.tile import TileContext
    from concourse.bass import broadcast_tensor_aps

    ops = _register_custom_ops()
    QUADSC, GSQ, RCPMUL = ops["quadsc"], ops["gsq"], ops["rcpmul"]
    RAMP_OPS = {("down", "down"): ops["ramp_dd"],
                ("down", "up"): ops["ramp_du"],
                ("up", "up"): ops["ramp_uu"]}

    f32 = mybir.dt.float32
    Alu = mybir.AluOpType
    Act = mybir.ActivationFunctionType

    n_tiles = b_core // P
    assert b_core % P == 0 and n_tiles % GROUP == 0
    w0, w1, w2 = (float(x) for x in smooth_w)

    nc = bacc.Bacc(None, target_bir_lowering=False, debug=False)
    patch_in = nc.dram_tensor("patch", [b_core, HW], f32, kind="ExternalInput")
    # consts: iota36 repeated n_tiles times, then (iota36 - 64) repeated
    consts_in = nc.dram_tensor("consts", [P, 2 * n_tiles * NBINS], f32,
                               kind="ExternalInput")
    # sobel Kronecker blocks for the PE path: [gx d0,dM,d7,sup,sub;
    # gy d0,dM,d7,sup,sub; identity] as 11 [128,128] blocks
    kmat_in = nc.dram_tensor("kmat", [P, 11 * 128], f32, kind="ExternalInput")
    wk_in = None
    if not wk_is_ones:
        wk_in = nc.dram_tensor("wk", [P, HW], f32, kind="ExternalInput")
    out_t = nc.dram_tensor("angle", [b_core], f32, kind="ExternalOutput")

    with TileContext(nc) as tc:
        with tc.tile_pool(name="pool", bufs=2) as pool, \
             tc.tile_pool(name="psumq", bufs=1, space="PSUM") as pq, \
             tc.tile_pool(name="persist", bufs=1) as pp:
            IOTA = pp.tile([P, 1, NBINS], f32)
            IOTA64 = pp.tile([P, 1, NBINS], f32)
            KM = pp.tile([P, 11 * 128], f32)
            nc.gpsimd.dma_start(KM[:], kmat_in[:])
            ID128 = KM[:, 10 * 128:11 * 128]
            WK = None
            if wk_in is not None:
                WK = pp.tile([P, HW], f32)
                nc.sync.dma_start(WK[:], wk_in[:])

            # 39 ramp slots per tile: [h_0..h_18, g_17..g_35, zero-pad]
            RAMP = pp.tile([P, n_tiles, 39], f32)
            ANG = pp.tile([P, n_tiles], f32)

            n_groups = n_tiles // GROUP

            def emit_hist(hg, hAW, hMW):
                for s in range(GROUP):
                    t = hg * GROUP + s
                    if t == 0:
                        continue
                    o_ap = hAW[:, s, :]
                    m_ap = hMW[:, s, :]
                    for i in range(NRAMP_OPS):
                        kA, jA, kB, jB = _pair_spec(i)
                        nc.vector._custom_dve(
                            RAMP_OPS[(kA, kB)],
                            out=RAMP[:, t, 2 * i:2 * i + 1]
                                .broadcast_to([P, HW]),
                            in0=o_ap, in1=m_ap, s0=jA, s1=jB,
                            accum_out=RAMP[:, t, 2 * i + 1:2 * i + 2])

            # ---- tail: ramps -> tents -> smoothing, argmax, refinement ----
            # T_k = 2nd difference of ramp sums; wrap bin T_0 = h_1 + g_35.
            # HEXT holds [T_35, T_0..T_35, T_0] (circular padding built in).
            HEXT = pp.tile([P, n_tiles, NBINS + 2], f32)
            SM = pp.tile([P, n_tiles, NBINS], f32)
            HS = pp.tile([P, n_tiles, NBINS], f32)
            VMAX = pp.tile([P, n_tiles, 1], f32)
            EQ = pp.tile([P, n_tiles, NBINS], f32)
            IDX = pp.tile([P, n_tiles, 1], f32)
            NV = {nm: [pp.tile([P, n_tiles, 1], f32, name=f"idxn_{nm}"),
                       pp.tile([P, n_tiles, 1], f32, name=f"wadj_{nm}"),
                       pp.tile([P, n_tiles, NBINS], f32, name=f"dif_{nm}"),
                       pp.tile([P, n_tiles, 1], f32, name=f"v_{nm}")]
                  for nm in ("p", "m")}
            NUM = pp.tile([P, n_tiles, 1], f32)
            SUMN = pp.tile([P, n_tiles, 1], f32)
            DEN = pp.tile([P, n_tiles, 1], f32)
            RECD = pp.tile([P, n_tiles, 1], f32)
            SCD = pp.tile([P, n_tiles, 1], f32)
            REF = pp.tile([P, n_tiles, 1], f32)
            out_view = out_t[:].rearrange("(t p) -> p t", p=P)

            def emit_tail(lo, hi):
                nt = hi - lo
                R = RAMP[:, lo:hi, :]
                HX = HEXT[:, lo:hi, :]
                # k=1..17: T_k = R[k-1] - 2R[k] + R[k+1]   -> HEXT[, k+1]
                nc.vector.tensor_tensor(HX[:, :, 2:19], R[:, :, 0:17],
                                        R[:, :, 2:19], Alu.add)
                nc.vector.scalar_tensor_tensor(
                    out=HX[:, :, 2:19], in0=R[:, :, 1:18], scalar=-2.0,
                    in1=HX[:, :, 2:19], op0=Alu.mult, op1=Alu.add)
                # k=18..35: T_k = R[k+1] - 2R[k+2] + R[k+3] -> HEXT[, k+1]
                nc.vector.tensor_tensor(HX[:, :, 19:37], R[:, :, 19:37],
                                        R[:, :, 21:39], Alu.add)
                nc.vector.scalar_tensor_tensor(
                    out=HX[:, :, 19:37], in0=R[:, :, 20:38], scalar=-2.0,
                    in1=HX[:, :, 19:37], op0=Alu.mult, op1=Alu.add)
                # T_0 = h_1 + g_35 = RAMP[1] + RAMP[37]
                nc.vector.tensor_tensor(HX[:, :, 1:2], R[:, :, 1:2],
                                        R[:, :, 37:38], Alu.add)
                # circular pads (single-source copies on ACT)
                nc.scalar.activation(HX[:, :, 0:1], HX[:, :, 36:37], Act.Copy)
                nc.scalar.activation(HX[:, :, 37:38], HX[:, :, 1:2], Act.Copy)

                sm = SM[:, lo:hi, :]
                nc.scalar.activation(sm, HX[:, :, 2:38], Act.Copy, scale=w2)
                nc.vector.scalar_tensor_tensor(
                    out=sm, in0=HX[:, :, 0:36], scalar=w0, in1=sm,
                    op0=Alu.mult, op1=Alu.add)
                hs = HS[:, lo:hi, :]
                nc.vector.scalar_tensor_tensor(
                    out=hs, in0=HX[:, :, 1:37], scalar=w1, in1=sm,
                    op0=Alu.mult, op1=Alu.add)

                vmax = VMAX[:, lo:hi, :]
                nc.vector.tensor_reduce(vmax, hs, mybir.AxisListType.X,
                                        Alu.max)
                eq = EQ[:, lo:hi, :]
                hs_b, vmax_b = broadcast_tensor_aps(hs, vmax)
                nc.vector.tensor_tensor(eq, hs_b, vmax_b, Alu.is_equal)
                nc.vector.tensor_tensor(
                    eq, eq, IOTA64[:].broadcast_to([P, nt, NBINS]), Alu.mult)
                idx = IDX[:, lo:hi, :]
                nc.vector.tensor_reduce(idx, eq, mybir.AxisListType.X,
                                        Alu.min)
                nc.scalar.activation(idx, idx, Act.Copy, bias=64.0)

                def neighbor_value(shift, wrap_thr, wrap_add, nm):
                    IDXN, WADJ, DIF, V = (x[:, lo:hi, :] for x in NV[nm])
                    nc.scalar.activation(IDXN, idx, Act.Copy,
                                         bias=float(shift))
                    if wrap_add < 0:
                        nc.vector.tensor_scalar(WADJ, IDXN, wrap_thr,
                                                float(wrap_add), Alu.is_gt,
                                                Alu.mult)
                    else:
                        nc.vector.tensor_scalar(WADJ, IDXN, wrap_thr,
                                                float(wrap_add), Alu.is_lt,
                                                Alu.mult)
                    nc.vector.tensor_tensor(IDXN, IDXN, WADJ, Alu.add)
                    iota_b, idxn_b = broadcast_tensor_aps(
                        IOTA[:].broadcast_to([P, nt, NBINS]), IDXN)
                    nc.vector.tensor_tensor(DIF, iota_b, idxn_b,
                                            Alu.subtract)
                    nc.vector.tensor_scalar(DIF, DIF, 0.0, None, Alu.is_equal)
                    nc.vector.tensor_tensor(DIF, DIF, hs, Alu.mult)
                    nc.vector.tensor_reduce(V, DIF, mybir.AxisListType.X,
                                            Alu.add)
                    return V

                VP = neighbor_value(+1, 35.5, -36.0, "p")
                VM = neighbor_value(-1, -0.5, +36.0, "m")

                num = NUM[:, lo:hi, :]
                nc.vector.tensor_tensor(num, VP, VM, Alu.subtract)
                sumn = SUMN[:, lo:hi, :]
                nc.vector.tensor_tensor(sumn, VP, VM, Alu.add)
                den = DEN[:, lo:hi, :]
                nc.scalar.activation(den, vmax, Act.Copy, scale=2.0)
                nc.vector.tensor_tensor(den, den, sumn, Alu.subtract)
                recd = RECD[:, lo:hi, :]
                nc.vector.reciprocal_approx_accurate(recd, den,
                                                     SCD[:, lo:hi, :])
                ref = REF[:, lo:hi, :]
                nc.vector.scalar_tensor_tensor(
                    out=ref, in0=num, scalar=0.5, in1=recd,
                    op0=Alu.mult, op1=Alu.mult)
                nc.vector.tensor_tensor(ref, idx, ref, Alu.add)
                nc.vector.tensor_scalar(ANG[:, lo:hi], ref[:, :, 0],
                                        -2.0 * PI / NBINS, PI,
                                        Alu.mult, Alu.add)
                nc.sync.dma_start(out_view[:, lo:hi], ANG[:, lo:hi])


            pending = None  # (g, AW, MW) - hist deferred one group so the
            # next group's phases are already queued when DVE grinds the
            # ramp ops (keeps every engine fed across group boundaries)
            for g in range(n_groups):
                tiles = range(g * GROUP, (g + 1) * GROUP)
                AW = pool.tile([P, GROUP, HW], f32, tag="aw", name=f"aw{g}")
                MW = pool.tile([P, GROUP, HW], f32, tag="mw", name=f"mw{g}")
                slot = {}
                # ---- phase A: sobel on PE (transpose + banded Kronecker
                # matmuls), mag (ACT sqrt table) ----
                for t in tiles:
                    s = t % GROUP
                    X = pool.tile([P, HW], f32, tag="x", bufs=3, name=f"x{t}")
                    nc.sync.dma_start(X[:], patch_in[t * P:(t + 1) * P, :])

                    if t == 0:
                        # tile 0 on the otherwise-idle DVE so its pipeline
                        # starts ~15us before the PE chain warms up
                        X3 = X.rearrange("p (r c) -> p r c", c=PATCH)
                        SV = pool.tile([P, HW], f32, tag="sv", name=f"sv{t}")
                        T1 = pool.tile([P, HW], f32, tag="t1", bufs=1,
                                       name=f"t1{t}")
                        nc.vector.tensor_tensor(
                            T1[:, 0:992], X[:, 0:992], X[:, 32:1024], Alu.add)
                        nc.vector.tensor_tensor(
                            SV[:, 32:992], T1[:, 0:960], T1[:, 32:992],
                            Alu.add)
                        SVE = SV.rearrange("p (r c) -> p r c", c=PATCH)[:, 0:32:31, :]
                        T1E = T1.rearrange("p (r c) -> p r c", c=PATCH)[:, 0:31:30, :]
                        nc.vector.scalar_tensor_tensor(
                            out=SVE, in0=X3[:, 0:32:31, :], scalar=2.0,
                            in1=T1E, op0=Alu.mult, op1=Alu.add)
                        SV3 = SV.rearrange("p (r c) -> p r c", c=PATCH)
                        GX = pool.tile([P, HW], f32, tag="gx", bufs=4,
                                       name=f"gx{t}")
                        GX3 = GX.rearrange("p (r c) -> p r c", c=PATCH)
                        nc.vector.tensor_tensor(
                            GX[:, 1:1023], SV[:, 2:1024], SV[:, 0:1022],
                            Alu.subtract)
                        nc.vector.tensor_tensor(
                            GX3[:, :, 0:32:31], SV3[:, :, 1:32:30],
                            SV3[:, :, 0:31:30], Alu.subtract)
                        SH = pool.tile([P, HW], f32, tag="sh", bufs=4,
                                       name=f"sh{t}")
                        SH3 = SH.rearrange("p (r c) -> p r c", c=PATCH)
                        nc.vector.scalar_tensor_tensor(
                            out=SH[:, 1:1023], in0=X[:, 1:1023], scalar=2.0,
                            in1=X[:, 0:1022], op0=Alu.mult, op1=Alu.add)
                        nc.vector.tensor_tensor(
                            SH[:, 1:1023], SH[:, 1:1023], X[:, 2:1024],
                            Alu.add)
                        nc.vector.scalar_tensor_tensor(
                            out=SH3[:, :, 0:32:31], in0=X3[:, :, 0:32:31],
                            scalar=3.0, in1=X3[:, :, 1:31:29], op0=Alu.mult,
                            op1=Alu.add)
                        GY = pool.tile([P, HW], f32, tag="gy", bufs=4,
                                       name=f"gy{t}")
                        nc.vector.tensor_tensor(
                            GY[:, 32:992], SH[:, 64:1024], SH[:, 0:960],
                            Alu.subtract)
                        GYE = GY.rearrange("p (r c) -> p r c", c=PATCH)[:, 0:32:31, :]
                        SH3e = SH.rearrange("p (r c) -> p r c", c=PATCH)
                        nc.vector.tensor_tensor(
                            GYE, SH3e[:, 1:32:30, :], SH3e[:, 0:31:30, :],
                            Alu.subtract)
                        nc.scalar.activation(GX[:], GX[:], Act.Copy,
                                             bias=1e-18)
                        if WK is not None:
                            nc.vector.tensor_tensor(GX[:], GX[:], WK[:],
                                                    Alu.mult)
                            nc.vector.tensor_tensor(GY[:], GY[:], WK[:],
                                                    Alu.mult)
                        G2 = pool.tile([P, HW], f32, tag="g2", name=f"g2{t}")
                        nc.vector._custom_dve(GSQ, out=G2[:], in0=GX[:],
                                              in1=GY[:])
                        nc.scalar.activation(MW[:, s, :], G2[:], Act.Sqrt)
                        RC = pool.tile([P, HW], f32, tag="rc", bufs=4,
                                       name=f"rc{t}")
                        nc.vector.reciprocal_approx_fast(RC[:], GX[:])
                        # run tile 0's phase B + histogram inline: ~30us of
                        # immediate DVE work covering the PE/ACT warmup
                        U0 = pool.tile([P, HW], f32, tag="sv", name="u0w")
                        nc.vector.tensor_tensor(U0[:], GY[:], RC[:], Alu.mult)
                        AU0 = pool.tile([P, HW], f32, tag="sh", bufs=4,
                                        name="au0w")
                        nc.scalar.activation(AU0[:], U0[:], Act.Arctan)
                        nc.vector._custom_dve(QUADSC, out=AW[:, s, :],
                                              in0=AU0[:], in1=GX[:],
                                              s0=18.0 / PI, s1=18.0,
                                              imm2=36.0)
                        for i in range(NRAMP_OPS):
                            kA, jA, kB, jB = _pair_spec(i)
                            nc.vector._custom_dve(
                                RAMP_OPS[(kA, kB)],
                                out=RAMP[:, t, 2 * i:2 * i + 1]
                                    .broadcast_to([P, HW]),
                                in0=AW[:, s, :], in1=MW[:, s, :],
                                s0=jA, s1=jB,
                                accum_out=RAMP[:, t, 2 * i + 1:2 * i + 2])
                        continue

                    # X^T chunks via PE transpose -> PSUM -> SBUF (ACT copy)
                    XTp = pq.tile([P, HW], f32, tag="xtp", name=f"xtp{t}")
                    for ch in range(8):
                        nc.tensor.transpose(
                            XTp[:, ch * 128:(ch + 1) * 128],
                            X[:, ch * 128:(ch + 1) * 128], ID128)
                    XT = pool.tile([P, HW], f32, tag="xt", bufs=2,
                                   name=f"xt{t}")
                    nc.scalar.activation(XT[:], XTp[:], Act.Copy)

                    # G[b, px'] = sum_px X[b,px] K[px,px']: lhsT = X^T chunk,
                    # rhs = Kronecker block; accumulate over the <=3
                    # contributing input chunks per output chunk.
                    GXp = pq.tile([P, HW], f32, tag="gxp", name=f"gxp{t}")
                    GYp = pq.tile([P, HW], f32, tag="gyp", name=f"gyp{t}")
                    for c in range(8):
                        dlo, dhi = max(0, c - 1), min(7, c + 1)
                        for d in range(dlo, dhi + 1):
                            lhsT = XT[:, d * 128:(d + 1) * 128]
                            for gbase, Gp in ((0, GXp), (5, GYp)):
                                if c == d:
                                    blk = gbase + (0 if d == 0 else
                                                   (2 if d == 7 else 1))
                                elif c == d + 1:
                                    blk = gbase + 3
                                else:
                                    blk = gbase + 4
                                nc.tensor.matmul(
                                    Gp[:, c * 128:(c + 1) * 128], lhsT,
                                    KM[:, blk * 128:(blk + 1) * 128],
                                    start=(d == dlo), stop=(d == dhi))

                    # PSUM -> SBUF on ACT (frees PSUM fast; +1e-18 folded in)
                    GX = pool.tile([P, HW], f32, tag="gx", bufs=4,
                                   name=f"gx{t}")
                    nc.scalar.activation(GX[:], GXp[:], Act.Copy, bias=1e-18)
                    GY = pool.tile([P, HW], f32, tag="gy", bufs=4,
                                   name=f"gy{t}")
                    nc.scalar.activation(GY[:], GYp[:], Act.Copy)

                    if WK is not None:
                        nc.vector.tensor_tensor(GX[:], GX[:], WK[:], Alu.mult)
                        nc.vector.tensor_tensor(GY[:], GY[:], WK[:], Alu.mult)

                    # g2 = gx^2 + gy^2 fused on DVE (exact fp32; eps dropped:
                    # sqrt(0)=0 -> zero tent weight, harmless)
                    G2 = pool.tile([P, HW], f32, tag="g2", name=f"g2{t}")
                    nc.vector._custom_dve(GSQ, out=G2[:], in0=GX[:],
                                          in1=GY[:])
                    # mag feeds only the tent weights; plain ACT sqrt is ample
                    nc.scalar.activation(MW[:, s, :], G2[:], Act.Sqrt)

                    RC = pool.tile([P, HW], f32, tag="rc", bufs=4,
                                   name=f"rc{t}")
                    nc.vector.reciprocal_approx_fast(RC[:], GX[:])
                    slot[t] = (GX, RC, GY)

                # ---- phase B: orientation (ACT arctan table) ----
                slot_b = {}
                for t in tiles:
                    if t not in slot:
                        continue
                    GXS, RC, GY = slot[t]
                    U = pool.tile([P, HW], f32, tag="sv", name=f"u{t}")
                    nc.vector.tensor_tensor(U[:], GY[:], RC[:], Alu.mult)
                    AU = pool.tile([P, HW], f32, tag="sh", bufs=4,
                                   name=f"au{t}")
                    nc.scalar.activation(AU[:], U[:], Act.Arctan)
                    slot_b[t] = (GXS, AU)
                for t in tiles:
                    if t not in slot_b:
                        continue
                    s = t % GROUP
                    GXS, AU = slot_b[t]
                    # fused scale+quadrant: o = (18/pi)*au + {0|18|36}
                    nc.vector._custom_dve(QUADSC, out=AW[:, s, :],
                                          in0=AU[:], in1=GXS[:],
                                          s0=18.0 / PI, s1=18.0, imm2=36.0)

                if g == 1:
                    # tail-only constants + pad memset (needed from the
                    # first tail segment onward)
                    nc.sync.dma_start(
                        IOTA[:, 0, :], consts_in[:, 0:NBINS])
                    nc.sync.dma_start(
                        IOTA64[:, 0, :],
                        consts_in[:, n_tiles * NBINS:n_tiles * NBINS + NBINS])
                    nc.vector.memset(RAMP[:, :, 38:39], 0.0)
                if pending is not None:
                    emit_hist(*pending)
                    if pending[0] == (n_tiles // 2) // GROUP - 1:
                        emit_tail(0, n_tiles // 2)
                    elif pending[0] == n_groups - 2:
                        emit_tail(n_tiles // 2, n_tiles - GROUP)
                pending = (g, AW, MW)

            emit_hist(*pending)
            emit_tail(n_tiles - GROUP, n_tiles)


    nc.compile()
    return nc


def _get_built(b_core, smooth_w, wk_is_ones):
    key = (b_core, tuple(float(x) for x in smooth_w), bool(wk_is_ones))
    if key not in _BUILD_CACHE:
        _BUILD_CACHE[key] = _build(b_core, smooth_w, wk_is_ones)
    return _BUILD_CACHE[key]


# --------------------------------------------------------------------------
# host entry point
# --------------------------------------------------------------------------
def kernel(patch, weight_kernel, smooth_w):
    from concourse import bass_utils

    patch = np.ascontiguousarray(np.asarray(patch, dtype=np.float32))
    weight_kernel = np.asarray(weight_kernel, dtype=np.float32)
    smooth_w = np.asarray(smooth_w, dtype=np.float32)

    B = patch.shape[0]
    assert B % (N_CORES * P) == 0, f"B={B} not divisible by {N_CORES * P}"
    b_core = B // N_CORES
    n_tiles = b_core // P

    wk_is_ones = bool(np.all(weight_kernel == 1.0))
    nc = _get_built(b_core, smooth_w, wk_is_ones)

    x = patch.reshape(N_CORES, b_core, HW)

    iota = np.tile(np.arange(NBINS, dtype=np.float32), n_tiles)
    consts_row = np.concatenate([iota, iota - 64.0]).astype(np.float32)
    consts = np.ascontiguousarray(
        np.broadcast_to(consts_row, (P, consts_row.size)))

    kmat = _make_kmat()

    in_maps = []
    for i in range(N_CORES):
        m = {"patch": np.ascontiguousarray(x[i]), "consts": consts,
             "kmat": kmat}
        if not wk_is_ones:
            m["wk"] = np.ascontiguousarray(
                np.broadcast_to(weight_kernel.reshape(-1), (P, HW)))
        in_maps.append(m)

    res = bass_utils.run_bass_kernel_spmd(nc, in_maps,
                                          core_ids=list(range(N_CORES)))
    out = np.concatenate([r["angle"] for r in res.results])
    return out.astype(np.float32)
